# revision 8
# baseline (speedup 1.0000x reference)
"""Trainium2 Bass kernel for nn_CrossAttention (B=2, T=V=4096, 16 heads, d=64).

Math: the reference einsums contract the k/v group axis g, so
  weight = softmax((x@Wq) @ (adj @ sum_g Wk_g)^T / sqrt(64))
  out    = (weight @ (adj @ sum_g Wv_g)) @ Wo + bo
The group fold (sum over g of Wk/Wv columns) is done host-side on the
weights; all tensor-sized compute runs on device.

Sharding: 8 cores = (batch b, quarter of T). Each core takes t-rows
[tq*1024, (tq+1)*1024) of batch b, needs adj[b] (redundant across the 4
cores of the same b), and writes its own out slice. No collectives.

Device pipeline per core (all fp32):
  B: stream adj[b] in 256-row stripes -> PE-transpose -> adjT -> K^T
     ([64,4096], zero-padded to 128 partitions) and V~ ([v,65] tiles,
     col 64 = ones so P@V also yields softmax denominators).
  C: same for x slice -> q^T per head, zero-padded to K=128 so every
     matmul runs in the PE's (128,128) tile mode (no mode switches).
  D: per (t-half, 4-head group): for each of 32 v-blocks, 4 S^T matmuls
     into one [128,2048] PSUM tile, a single Exp on ACT (scale=1/8
     folded in), then 4 P@V matmuls accumulating O^T[65,512] per head.
     Row 64 of O^T = softmax sum; reciprocal + broadcast-multiply
     normalizes into attnT.
  E: out-proj from attnT with Wo, bias add, DMA out.
"""

import numpy as np

import concourse.bass as bass
import concourse.tile as tile
from concourse import bacc, mybir
from concourse.masks import make_identity

F32 = mybir.dt.float32
F32R = mybir.dt.float32r


def _r(ap):
    return ap.bitcast(F32R)

# Problem constants (hardcoded per the harness contract).
B = 2
T = 4096
V = 4096
E = 1024     # n_embd
HID = 1024   # n_hidden
NH = 16
DH = 64
G = 4
N_CORES = 8
T_CORE = (B * T) // N_CORES  # 1024 t-rows per core
P = 128

# Tiling parameters.
T_TILE = 512          # t-columns per attention tile (fp32 matmul N max)
HPG = 4               # heads per group (4 S banks + 4 O banks = 8 PSUM banks)
ROW_G = 256           # rows per transpose stripe in build phases
SCALE = 1.0 / 8.0     # 1/sqrt(DH)


def build_nc():
    """Build the per-core Bass program (same program on all 8 cores)."""
    EB = E // P                # 8  e-blocks
    DB = HID // P              # 8  dq-blocks
    NVB = V // P               # 32 v-blocks
    NTT = T_CORE // T_TILE     # 2  t-halves
    NHG = NH // HPG            # 4  head groups
    GC = ROW_G // P            # 2  128-row chunks per stripe
    NSTRIPE_V = V // ROW_G     # 16
    NSTRIPE_T = T_TILE // ROW_G  # 2 stripes per t-half

    nc = bacc.Bacc("TRN2", target_bir_lowering=False, debug=False,
                   num_devices=N_CORES)

    x_sl = nc.declare_dram_parameter("x_sl", [T_CORE, E], F32, isOutput=False)
    adj_b = nc.declare_dram_parameter("adj_b", [V, E], F32, isOutput=False)
    Wq = nc.declare_dram_parameter("Wq", [E, HID], F32R, isOutput=False)
    bq = nc.declare_dram_parameter("bq", [HID], F32, isOutput=False)
    Wk = nc.declare_dram_parameter("Wk", [E, DH], F32R, isOutput=False)
    bk = nc.declare_dram_parameter("bk", [DH], F32, isOutput=False)
    Wv = nc.declare_dram_parameter("Wv", [E, DH], F32R, isOutput=False)
    bv = nc.declare_dram_parameter("bv", [DH], F32, isOutput=False)
    Wo = nc.declare_dram_parameter("Wo", [HID, HID], F32R, isOutput=False)
    bo = nc.declare_dram_parameter("bo", [HID], F32, isOutput=False)
    out_sl = nc.declare_dram_parameter("out_sl", [T_CORE, HID], F32,
                                       isOutput=True)
    # DRAM bounce buffer for partition-broadcasting softmax reciprocals.
    sums_dram = nc.dram_tensor("sums_scratch", [NH, T_CORE], F32)

    def bcast_ap(param, n_part, n_free):
        a = param[:] if not isinstance(param, bass.AP) else param
        return bass.AP(tensor=a.tensor, offset=a.offset,
                       ap=[[0, n_part]] + list(a.ap))

    from contextlib import ExitStack
    with tile.TileContext(nc, pool_alloc_mode="queue") as tc, ExitStack() as st:
        consts = st.enter_context(tc.tile_pool(name="consts", bufs=1))
        persist = st.enter_context(tc.tile_pool(name="persist", bufs=1))

        ident = consts.tile([P, P], F32)
        make_identity(nc, ident[:])
        bq_sb = consts.tile([P, DB], F32)
        nc.sync.dma_start(bq_sb[:], bq.rearrange("(db dp) -> dp db", dp=P))
        bk_sb = consts.tile([DH, 1], F32)
        nc.sync.dma_start(bk_sb[:], bk.rearrange("(a one) -> a one", one=1))
        bvb = consts.tile([P, DH], F32)
        nc.gpsimd.dma_start(bvb[:], bcast_ap(bv, P, DH))
        bob = consts.tile([P, HID], F32)
        nc.gpsimd.dma_start(bob[:], bcast_ap(bo, P, HID))

        # Persistent operands of the attention phase.
        kT = persist.tile([P, V], F32R)          # K^T, rows 64..127 zero
        vt = persist.tile([P, NVB, DH + 1], F32R)  # V~ per v-block + ones col
        qT = persist.tile([P, NH, T_CORE], F32R)   # q^T per head, zero-padded
        attnT = persist.tile([P, DB, T_CORE], F32R)  # normalized O^T
        nc.gpsimd.memset(kT[DH:P, :].bitcast(F32), 0.0)
        nc.gpsimd.memset(qT[DH:P, :, :].bitcast(F32), 0.0)
        nc.gpsimd.memset(vt[:, :, DH:DH + 1].bitcast(F32), 1.0)

        # ---- Phase B: K^T and V~ from adj ----
        with (
            tc.tile_pool(name="bwork", bufs=2) as bw,
            tc.tile_pool(name="bw1", bufs=1) as bw1,
            tc.tile_pool(name="bpsum", bufs=2, space="PSUM") as bp,
        ):
            # Wk padded to 128 cols so the K-proj output is [128, N].
            Wk_sb = bw1.tile([P, EB, P], F32R)
            nc.gpsimd.memset(Wk_sb[:, :, DH:P].bitcast(F32), 0.0)
            nc.sync.dma_start(Wk_sb[:, :, 0:DH],
                             Wk.rearrange("(eb ep) d -> ep eb d", ep=P))
            Wv_sb = bw1.tile([P, EB, DH], F32R)
            nc.sync.dma_start(Wv_sb[:],
                             Wv.rearrange("(eb ep) d -> ep eb d", ep=P))

            for sv in range(NSTRIPE_V):
                r0 = sv * ROW_G
                adj_in = bw.tile([P, GC, E], F32, tag="row_in")
                nc.sync.dma_start(
                    adj_in[:],
                    adj_b[r0:r0 + ROW_G, :].rearrange("(c p) e -> p c e", p=P))
                aT = bw.tile([P, EB, ROW_G], F32R, tag="aT")
                for eb in range(EB):
                    for cc in range(GC):
                        ptr = bp.tile([P, P], F32, tag="ptr")
                        nc.tensor.transpose(
                            ptr[:], adj_in[:, cc, eb * P:(eb + 1) * P], ident[:])
                        nc.vector.tensor_copy(aT[:, eb, cc * P:(cc + 1) * P],
                                              ptr[:])
                # K^T columns for this stripe.
                pk = bp.tile([P, ROW_G], F32, tag="pk")
                for eb in range(EB):
                    nc.tensor.matmul(pk[:], Wk_sb[:, eb, :], aT[:, eb, :],
                                     start=(eb == 0), stop=(eb == EB - 1))
                nc.vector.tensor_scalar_add(kT[0:DH, r0:r0 + ROW_G],
                                            pk[0:DH, :], bk_sb[:])
                # V~ rows for this stripe.
                for cc in range(GC):
                    vb = (r0 + cc * P) // P
                    pv = bp.tile([P, DH], F32, tag="pv")
                    for eb in range(EB):
                        nc.tensor.matmul(pv[:], aT[:, eb, cc * P:(cc + 1) * P],
                                         Wv_sb[:, eb, :],
                                         start=(eb == 0), stop=(eb == EB - 1))
                    nc.vector.tensor_add(vt[:, vb, 0:DH], pv[:], bvb[:])

        # ---- Phase C: q^T from x ----
        with (
            tc.tile_pool(name="cwork", bufs=2) as cw,
            tc.tile_pool(name="cw1", bufs=1) as cw1,
            tc.tile_pool(name="cpsum", bufs=2, space="PSUM") as cp,
        ):
            for half in range(2):
                db_lo = half * (DB // 2)
                Wq_sb = cw1.tile([P, EB, (DB // 2) * P], F32R, tag="wq")
                nc.sync.dma_start(
                    Wq_sb[:],
                    Wq[:, db_lo * P:(db_lo + DB // 2) * P]
                    .rearrange("(eb ep) d -> ep eb d", ep=P))
                for tt in range(NTT):
                    xT = cw1.tile([P, EB, T_TILE], F32R, tag="xT")
                    for st in range(NSTRIPE_T):
                        r0 = tt * T_TILE + st * ROW_G
                        x_in = cw.tile([P, GC, E], F32, tag="x_in")
                        nc.sync.dma_start(
                            x_in[:],
                            x_sl[r0:r0 + ROW_G, :]
                            .rearrange("(c p) e -> p c e", p=P))
                        for eb in range(EB):
                            for cc in range(GC):
                                ptr = cp.tile([P, P], F32, tag="ptr")
                                nc.tensor.transpose(
                                    ptr[:], x_in[:, cc, eb * P:(eb + 1) * P],
                                    ident[:])
                                nc.vector.tensor_copy(
                                    xT[:, eb, st * ROW_G + cc * P:
                                       st * ROW_G + (cc + 1) * P], ptr[:])
                    ts0 = tt * T_TILE
                    for dbr in range(DB // 2):
                        db = db_lo + dbr
                        pq = cp.tile([P, T_TILE], F32, tag="pq")
                        for eb in range(EB):
                            nc.tensor.matmul(
                                pq[:], Wq_sb[:, eb, dbr * P:(dbr + 1) * P],
                                xT[:, eb, :],
                                start=(eb == 0), stop=(eb == EB - 1))
                        # head 2*db from partitions 0..63 (same-lane copy)
                        nc.vector.tensor_scalar_add(
                            qT[0:DH, 2 * db, ts0:ts0 + T_TILE],
                            pq[0:DH, :], bq_sb[0:DH, db:db + 1])
                        # head 2*db+1 from partitions 64..127 (via DMA)
                        qtmp = cw.tile([P, T_TILE], F32R, tag="qtmp")
                        nc.vector.tensor_scalar_add(
                            qtmp[DH:P, :], pq[DH:P, :], bq_sb[DH:P, db:db + 1])
                        nc.gpsimd.dma_start(
                            qT[0:DH, 2 * db + 1, ts0:ts0 + T_TILE],
                            qtmp[DH:P, :])

        # ---- Phase D: attention ----
        with (
            tc.tile_pool(name="dwork", bufs=2) as dw,
            tc.tile_pool(name="dnorm", bufs=2) as dn,
            tc.tile_pool(name="dpsum", bufs=1, space="PSUM") as dps,
            tc.tile_pool(name="opsum", bufs=HPG, space="PSUM") as ops,
        ):
            for tt in range(NTT):
                ts0 = tt * T_TILE
                for hg in range(NHG):
                    heads = [hg * HPG + i for i in range(HPG)]
                    O4 = [ops.tile([P, T_TILE], F32, tag="O4", name=f"O4_{i}") for i in range(len(heads))]
                    for vb in range(NVB):
                        S4 = dps.tile([P, HPG * T_TILE], F32, tag="S4")
                        for hi, h in enumerate(heads):
                            nc.tensor.matmul(
                                S4[:, hi * T_TILE:(hi + 1) * T_TILE],
                                kT[:, vb * P:(vb + 1) * P],
                                qT[:, h, ts0:ts0 + T_TILE],
                                start=True, stop=True)
                        P4 = dw.tile([P, HPG * T_TILE], F32R, tag="P4")
                        nc.scalar.activation(
                            P4[:], S4[:], mybir.ActivationFunctionType.Exp,
                            scale=SCALE)
                        for hi, h in enumerate(heads):
                            nc.tensor.matmul(
                                O4[hi][0:DH + 1, :], vt[:, vb, :],
                                P4[:, hi * T_TILE:(hi + 1) * T_TILE],
                                start=(vb == 0), stop=(vb == NVB - 1))
                    # Normalize: row DH of O4 holds the softmax denominator.
                    for hi, h in enumerate(heads):
                        recip = dn.tile([P, T_TILE], F32, tag="recip")
                        nc.vector.reciprocal(recip[DH:DH + 1, :],
                                             O4[hi][DH:DH + 1, :])
                        nc.sync.dma_start(sums_dram[h:h + 1, ts0:ts0 + T_TILE],
                                          recip[DH:DH + 1, :])
                        rbc = dn.tile([DH, T_TILE], F32, tag="rbc")
                        nc.gpsimd.dma_start(
                            rbc[:], bcast_ap(sums_dram[h, ts0:ts0 + T_TILE],
                                             DH, T_TILE))
                        db = h // 2
                        if h % 2 == 0:
                            nc.vector.tensor_mul(
                                attnT[0:DH, db, ts0:ts0 + T_TILE],
                                O4[hi][0:DH, :], rbc[:])
                        else:
                            nrm = dn.tile([DH, T_TILE], F32, tag="nrm")
                            nc.vector.tensor_mul(nrm[:], O4[hi][0:DH, :],
                                                 rbc[:])
                            nc.gpsimd.dma_start(
                                attnT[DH:P, db, ts0:ts0 + T_TILE], nrm[:])

        # ---- Phase E: output projection ----
        with (
            tc.tile_pool(name="ework", bufs=3) as ew,
            tc.tile_pool(name="ew1", bufs=1) as ew1,
            tc.tile_pool(name="epsum", bufs=2, space="PSUM") as ep,
        ):
            Wo_sb = ew1.tile([P, DB, HID], F32R)
            nc.sync.dma_start(Wo_sb[:],
                             Wo.rearrange("(kb kp) e -> kp kb e", kp=P))
            for tc_i in range(T_CORE // P):
                for eh in range(HID // T_TILE):
                    po = ep.tile([P, T_TILE], F32, tag="po")
                    for kb in range(DB):
                        nc.tensor.matmul(
                            po[:], attnT[:, kb, tc_i * P:(tc_i + 1) * P],
                            Wo_sb[:, kb, eh * T_TILE:(eh + 1) * T_TILE],
                            start=(kb == 0), stop=(kb == DB - 1))
                    ot = ew.tile([P, T_TILE], F32, tag="ot")
                    nc.vector.tensor_add(
                        ot[:], po[:], bob[:, eh * T_TILE:(eh + 1) * T_TILE])
                    nc.sync.dma_start(
                        out_sl[tc_i * P:(tc_i + 1) * P,
                               eh * T_TILE:(eh + 1) * T_TILE], ot[:])

    nc.compile()
    return nc


_NC = None


def _get_nc():
    global _NC
    if _NC is None:
        _NC = build_nc()
    return _NC


def kernel(x, adj, Wq, bq, Wk, bk, Wv, bv, Wo, bo):
    x = np.asarray(x, np.float32)
    adj = np.asarray(adj, np.float32)
    Wq_f = np.ascontiguousarray(np.asarray(Wq, np.float32))
    bq_f = np.ascontiguousarray(np.asarray(bq, np.float32))
    Wk_f = np.ascontiguousarray(
        np.asarray(Wk, np.float32).reshape(E, G, DH).sum(axis=1))
    bk_f = np.ascontiguousarray(
        np.asarray(bk, np.float32).reshape(G, DH).sum(axis=0))
    Wv_f = np.ascontiguousarray(
        np.asarray(Wv, np.float32).reshape(E, G, DH).sum(axis=1))
    bv_f = np.ascontiguousarray(
        np.asarray(bv, np.float32).reshape(G, DH).sum(axis=0))
    Wo_f = np.ascontiguousarray(np.asarray(Wo, np.float32))
    bo_f = np.ascontiguousarray(np.asarray(bo, np.float32))

    nc = _get_nc()
    in_maps = []
    for c in range(N_CORES):
        b = c // (N_CORES // B)
        tq = c % (N_CORES // B)
        in_maps.append({
            "x_sl": np.ascontiguousarray(
                x[b, tq * T_CORE:(tq + 1) * T_CORE, :]),
            "adj_b": np.ascontiguousarray(adj[b]),
            "Wq": Wq_f, "bq": bq_f, "Wk": Wk_f, "bk": bk_f,
            "Wv": Wv_f, "bv": bv_f, "Wo": Wo_f, "bo": bo_f,
        })

    from concourse.bass_utils import run_bass_kernel_spmd
    res = run_bass_kernel_spmd(nc, in_maps, list(range(N_CORES)))

    out = np.empty((B, T, HID), np.float32)
    for c in range(N_CORES):
        b = c // (N_CORES // B)
        tq = c % (N_CORES // B)
        out[b, tq * T_CORE:(tq + 1) * T_CORE, :] = res.results[c]["out_sl"]
    return out


# revision 11
# speedup vs baseline: 1.7529x; 1.7529x over previous
"""Trainium2 Bass kernel for nn_CrossAttention (B=2, T=V=4096, 16 heads, d=64).

Math: the reference einsums contract the k/v group axis g, so
  weight = softmax((x@Wq) @ (adj @ sum_g Wk_g)^T / sqrt(64))
  out    = (weight @ (adj @ sum_g Wv_g)) @ Wo + bo
The group fold (sum over g of Wk/Wv columns) is done host-side on the
weights; all tensor-sized compute runs on device.

Sharding: 8 cores = (batch b, quarter of T). Each core takes t-rows
[tq*1024, (tq+1)*1024) of batch b, needs adj[b] (redundant across the 4
cores of the same b), and writes its own out slice. No collectives.

Device pipeline per core (all fp32):
  B: stream adj[b] in 256-row stripes -> PE-transpose -> adjT -> K^T
     ([64,4096], zero-padded to 128 partitions) and V~ ([v,65] tiles,
     col 64 = ones so P@V also yields softmax denominators).
  C: same for x slice -> q^T per head, zero-padded to K=128 so every
     matmul runs in the PE's (128,128) tile mode (no mode switches).
  D: per (t-half, 4-head group): for each of 32 v-blocks, 4 S^T matmuls
     into one [128,2048] PSUM tile, a single Exp on ACT (scale=1/8
     folded in), then 4 P@V matmuls accumulating O^T[65,512] per head.
     Row 64 of O^T = softmax sum; reciprocal + broadcast-multiply
     normalizes into attnT.
  E: out-proj from attnT with Wo, bias add, DMA out.
"""

import numpy as np

import concourse.bass as bass
import concourse.tile as tile
from concourse import bacc, mybir
from concourse.masks import make_identity

F32 = mybir.dt.float32
F32R = mybir.dt.float32r


def _r(ap):
    return ap.bitcast(F32R)

# Problem constants (hardcoded per the harness contract).
B = 2
T = 4096
V = 4096
E = 1024     # n_embd
HID = 1024   # n_hidden
NH = 16
DH = 64
G = 4
N_CORES = 8
T_CORE = (B * T) // N_CORES  # 1024 t-rows per core
P = 128

# Tiling parameters.
T_TILE = 512          # t-columns per attention tile (fp32 matmul N max)
HPG = 4               # heads per group (4 S banks + 4 O banks = 8 PSUM banks)
ROW_G = 256           # rows per transpose stripe in build phases
SCALE = 1.0 / 8.0     # 1/sqrt(DH)


def build_nc():
    """Build the per-core Bass program (same program on all 8 cores)."""
    EB = E // P                # 8  e-blocks
    DB = HID // P              # 8  dq-blocks
    NVB = V // P               # 32 v-blocks
    NTT = T_CORE // T_TILE     # 2  t-halves
    NHG = NH // HPG            # 4  head groups
    GC = ROW_G // P            # 2  128-row chunks per stripe
    NSTRIPE_V = V // ROW_G     # 16
    NSTRIPE_T = T_TILE // ROW_G  # 2 stripes per t-half

    nc = bacc.Bacc("TRN2", target_bir_lowering=False, debug=False,
                   num_devices=N_CORES)

    x_sl = nc.declare_dram_parameter("x_sl", [T_CORE, E], F32, isOutput=False)
    adj_b = nc.declare_dram_parameter("adj_b", [V, E], F32, isOutput=False)
    Wq = nc.declare_dram_parameter("Wq", [E, HID], F32R, isOutput=False)
    bq = nc.declare_dram_parameter("bq", [HID], F32, isOutput=False)
    Wk = nc.declare_dram_parameter("Wk", [E, DH], F32R, isOutput=False)
    bk = nc.declare_dram_parameter("bk", [DH], F32, isOutput=False)
    Wv = nc.declare_dram_parameter("Wv", [E, DH], F32R, isOutput=False)
    bv = nc.declare_dram_parameter("bv", [DH], F32, isOutput=False)
    Wo = nc.declare_dram_parameter("Wo", [HID, HID], F32R, isOutput=False)
    bo = nc.declare_dram_parameter("bo", [HID], F32, isOutput=False)
    out_sl = nc.declare_dram_parameter("out_sl", [T_CORE, HID], F32,
                                       isOutput=True)
    # DRAM bounce buffer for partition-broadcasting softmax reciprocals.
    sums_dram = nc.dram_tensor("sums_scratch", [NH, T_CORE], F32)

    def bcast_ap(param, n_part, n_free):
        a = param[:] if not isinstance(param, bass.AP) else param
        return bass.AP(tensor=a.tensor, offset=a.offset,
                       ap=[[0, n_part]] + list(a.ap))

    from contextlib import ExitStack
    with tile.TileContext(nc, pool_alloc_mode="queue") as tc, ExitStack() as st:
        consts = st.enter_context(tc.tile_pool(name="consts", bufs=1))
        persist = st.enter_context(tc.tile_pool(name="persist", bufs=1))

        ident = consts.tile([P, P], F32)
        make_identity(nc, ident[:])
        bq_sb = consts.tile([P, DB], F32)
        nc.sync.dma_start(bq_sb[:], bq.rearrange("(db dp) -> dp db", dp=P))
        bk_sb = consts.tile([DH, 1], F32)
        nc.sync.dma_start(bk_sb[:], bk.rearrange("(a one) -> a one", one=1))
        bvb = consts.tile([P, DH], F32)
        nc.gpsimd.dma_start(bvb[:], bcast_ap(bv, P, DH))
        bob = consts.tile([P, HID], F32)
        nc.gpsimd.dma_start(bob[:], bcast_ap(bo, P, HID))

        # Persistent operands of the attention phase.
        kT = persist.tile([P, V], F32R)          # K^T, rows 64..127 zero
        vt = persist.tile([P, NVB, DH + 1], F32R)  # V~ per v-block + ones col
        qT = persist.tile([P, NH, T_CORE], F32R)   # q^T per head, zero-padded
        attnT = persist.tile([P, DB, T_CORE], F32R)  # normalized O^T
        nc.gpsimd.memset(kT[DH:P, :].bitcast(F32), 0.0)
        nc.gpsimd.memset(qT[DH:P, :, :].bitcast(F32), 0.0)
        nc.gpsimd.memset(vt[:, :, DH:DH + 1].bitcast(F32), 1.0)

        # ---- Phase B: K^T and V~ from adj ----
        with (
            tc.tile_pool(name="bwork", bufs=2) as bw,
            tc.tile_pool(name="bw1", bufs=1) as bw1,
            tc.tile_pool(name="bpsum", bufs=2, space="PSUM") as bp,
        ):
            # Wk padded to 128 cols so the K-proj output is [128, N].
            Wk_sb = bw1.tile([P, EB, P], F32R)
            nc.gpsimd.memset(Wk_sb[:, :, DH:P].bitcast(F32), 0.0)
            nc.sync.dma_start(Wk_sb[:, :, 0:DH],
                             Wk.rearrange("(eb ep) d -> ep eb d", ep=P))
            Wv_sb = bw1.tile([P, EB, DH], F32R)
            nc.sync.dma_start(Wv_sb[:],
                             Wv.rearrange("(eb ep) d -> ep eb d", ep=P))

            for sv in range(NSTRIPE_V):
                r0 = sv * ROW_G
                adj_in = bw.tile([P, GC, E], F32, tag="row_in")
                nc.sync.dma_start(
                    adj_in[:],
                    adj_b[r0:r0 + ROW_G, :].rearrange("(c p) e -> p c e", p=P))
                aT = bw.tile([P, EB, ROW_G], F32R, tag="aT")
                for eb in range(EB):
                    for cc in range(GC):
                        ptr = bp.tile([P, P], F32, tag="ptr")
                        nc.tensor.transpose(
                            ptr[:], adj_in[:, cc, eb * P:(eb + 1) * P], ident[:])
                        nc.vector.tensor_copy(aT[:, eb, cc * P:(cc + 1) * P],
                                              ptr[:])
                # K^T columns for this stripe.
                pk = bp.tile([P, ROW_G], F32, tag="pk")
                for eb in range(EB):
                    nc.tensor.matmul(pk[:], Wk_sb[:, eb, :], aT[:, eb, :],
                                     start=(eb == 0), stop=(eb == EB - 1))
                nc.vector.tensor_scalar_add(kT[0:DH, r0:r0 + ROW_G],
                                            pk[0:DH, :], bk_sb[:])
                # V~ rows for this stripe.
                for cc in range(GC):
                    vb = (r0 + cc * P) // P
                    pv = bp.tile([P, DH], F32, tag="pv")
                    for eb in range(EB):
                        nc.tensor.matmul(pv[:], aT[:, eb, cc * P:(cc + 1) * P],
                                         Wv_sb[:, eb, :],
                                         start=(eb == 0), stop=(eb == EB - 1))
                    nc.vector.tensor_add(vt[:, vb, 0:DH], pv[:], bvb[:])

        # ---- Phase C: q^T from x ----
        with (
            tc.tile_pool(name="cwork", bufs=2) as cw,
            tc.tile_pool(name="cw1", bufs=1) as cw1,
            tc.tile_pool(name="cpsum", bufs=2, space="PSUM") as cp,
        ):
            for tt in range(NTT):
                for half in range(2):
                    db_lo = half * (DB // 2)
                    Wq_sb = cw1.tile([P, EB, (DB // 2) * P], F32R, tag="wq")
                    nc.sync.dma_start(
                        Wq_sb[:],
                        Wq[:, db_lo * P:(db_lo + DB // 2) * P]
                        .rearrange("(eb ep) d -> ep eb d", ep=P))
                    xT = cw1.tile([P, EB, T_TILE], F32R, tag="xT")
                    for st in range(NSTRIPE_T):
                        r0 = tt * T_TILE + st * ROW_G
                        x_in = cw.tile([P, GC, E], F32, tag="x_in")
                        nc.sync.dma_start(
                            x_in[:],
                            x_sl[r0:r0 + ROW_G, :]
                            .rearrange("(c p) e -> p c e", p=P))
                        for eb in range(EB):
                            for cc in range(GC):
                                ptr = cp.tile([P, P], F32, tag="ptr")
                                nc.tensor.transpose(
                                    ptr[:], x_in[:, cc, eb * P:(eb + 1) * P],
                                    ident[:])
                                nc.vector.tensor_copy(
                                    xT[:, eb, st * ROW_G + cc * P:
                                       st * ROW_G + (cc + 1) * P], ptr[:])
                    ts0 = tt * T_TILE
                    for dbr in range(DB // 2):
                        db = db_lo + dbr
                        pq = cp.tile([P, T_TILE], F32, tag="pq")
                        for eb in range(EB):
                            nc.tensor.matmul(
                                pq[:], Wq_sb[:, eb, dbr * P:(dbr + 1) * P],
                                xT[:, eb, :],
                                start=(eb == 0), stop=(eb == EB - 1))
                        # head 2*db from partitions 0..63 (same-lane copy)
                        nc.vector.tensor_scalar_add(
                            qT[0:DH, 2 * db, ts0:ts0 + T_TILE],
                            pq[0:DH, :], bq_sb[0:DH, db:db + 1])
                        # head 2*db+1 from partitions 64..127 (via DMA)
                        qtmp = cw.tile([P, T_TILE], F32R, tag="qtmp")
                        nc.vector.tensor_scalar_add(
                            qtmp[DH:P, :], pq[DH:P, :], bq_sb[DH:P, db:db + 1])
                        nc.gpsimd.dma_start(
                            qT[0:DH, 2 * db + 1, ts0:ts0 + T_TILE],
                            qtmp[DH:P, :])

        # ---- Phase D: attention ----
        # PSUM: 2 x S2[128,1024] (2 banks each, double-buffered) + 4 x O
        # banks = 8.  Each exp covers a 2-head [128,1024] tile so ACT
        # streams continuously while PE fills the other S2 buffer.
        with (
            tc.tile_pool(name="dwork", bufs=3) as dw,
            tc.tile_pool(name="dnorm", bufs=2) as dn,
            tc.tile_pool(name="dpsum", bufs=2, space="PSUM") as dps,
            tc.tile_pool(name="opsum", bufs=HPG, space="PSUM") as ops,
        ):
            for tt in range(NTT):
                ts0 = tt * T_TILE
                for hg in range(NHG):
                    heads = [hg * HPG + i for i in range(HPG)]
                    O4 = [ops.tile([P, T_TILE], F32, tag="O4",
                                   name=f"O4_{i}") for i in range(len(heads))]
                    for vb in range(NVB):
                        for pp in range(HPG // 2):
                            S2 = dps.tile([P, 2 * T_TILE], F32, tag="S2")
                            for h2 in range(2):
                                hi = pp * 2 + h2
                                nc.tensor.matmul(
                                    S2[:, h2 * T_TILE:(h2 + 1) * T_TILE],
                                    kT[:, vb * P:(vb + 1) * P],
                                    qT[:, heads[hi], ts0:ts0 + T_TILE],
                                    start=True, stop=True)
                            P2 = dw.tile([P, 2 * T_TILE], F32R, tag="P2")
                            nc.scalar.activation(
                                P2[:], S2[:],
                                mybir.ActivationFunctionType.Exp, scale=SCALE)
                            for h2 in range(2):
                                hi = pp * 2 + h2
                                nc.tensor.matmul(
                                    O4[hi][0:DH + 1, :], vt[:, vb, :],
                                    P2[:, h2 * T_TILE:(h2 + 1) * T_TILE],
                                    start=(vb == 0), stop=(vb == NVB - 1))
                    # Normalize: row DH of O4 holds the softmax denominator.
                    # Broadcast the sums row via DRAM, then a full-width
                    # reciprocal + multiply on 64 lanes.
                    for hi, h in enumerate(heads):
                        onorm = dn.tile([DH + 1, T_TILE], F32, tag="onorm")
                        nc.vector.tensor_copy(onorm[:], O4[hi][0:DH + 1, :])
                        nc.gpsimd.dma_start(
                            sums_dram[h:h + 1, ts0:ts0 + T_TILE],
                            onorm[DH:DH + 1, :])
                        sbc = dn.tile([DH, T_TILE], F32, tag="sbc")
                        nc.gpsimd.dma_start(
                            sbc[:], bcast_ap(sums_dram[h, ts0:ts0 + T_TILE],
                                             DH, T_TILE))
                        rec = dn.tile([DH, T_TILE], F32, tag="rec")
                        nc.vector.reciprocal(rec[:], sbc[:])
                        db = h // 2
                        if h % 2 == 0:
                            nc.vector.tensor_mul(
                                attnT[0:DH, db, ts0:ts0 + T_TILE],
                                onorm[0:DH, :], rec[:])
                        else:
                            nrm = dn.tile([DH, T_TILE], F32, tag="nrm")
                            nc.vector.tensor_mul(nrm[:], onorm[0:DH, :],
                                                 rec[:])
                            nc.gpsimd.dma_start(
                                attnT[DH:P, db, ts0:ts0 + T_TILE], nrm[:])

        # ---- Phase E: output projection ----
        with (
            tc.tile_pool(name="ework", bufs=3) as ew,
            tc.tile_pool(name="ew1", bufs=1) as ew1,
            tc.tile_pool(name="epsum", bufs=2, space="PSUM") as ep,
        ):
            Wo_sb = ew1.tile([P, DB, HID], F32R)
            nc.sync.dma_start(Wo_sb[:],
                             Wo.rearrange("(kb kp) e -> kp kb e", kp=P))
            for tc_i in range(T_CORE // P):
                for eh in range(HID // T_TILE):
                    po = ep.tile([P, T_TILE], F32, tag="po")
                    for kb in range(DB):
                        nc.tensor.matmul(
                            po[:], attnT[:, kb, tc_i * P:(tc_i + 1) * P],
                            Wo_sb[:, kb, eh * T_TILE:(eh + 1) * T_TILE],
                            start=(kb == 0), stop=(kb == DB - 1))
                    ot = ew.tile([P, T_TILE], F32, tag="ot")
                    nc.vector.tensor_add(
                        ot[:], po[:], bob[:, eh * T_TILE:(eh + 1) * T_TILE])
                    nc.sync.dma_start(
                        out_sl[tc_i * P:(tc_i + 1) * P,
                               eh * T_TILE:(eh + 1) * T_TILE], ot[:])

    nc.compile()
    return nc


_NC = None


def _get_nc():
    global _NC
    if _NC is None:
        _NC = build_nc()
    return _NC


def kernel(x, adj, Wq, bq, Wk, bk, Wv, bv, Wo, bo):
    x = np.asarray(x, np.float32)
    adj = np.asarray(adj, np.float32)
    Wq_f = np.ascontiguousarray(np.asarray(Wq, np.float32))
    bq_f = np.ascontiguousarray(np.asarray(bq, np.float32))
    Wk_f = np.ascontiguousarray(
        np.asarray(Wk, np.float32).reshape(E, G, DH).sum(axis=1))
    bk_f = np.ascontiguousarray(
        np.asarray(bk, np.float32).reshape(G, DH).sum(axis=0))
    Wv_f = np.ascontiguousarray(
        np.asarray(Wv, np.float32).reshape(E, G, DH).sum(axis=1))
    bv_f = np.ascontiguousarray(
        np.asarray(bv, np.float32).reshape(G, DH).sum(axis=0))
    Wo_f = np.ascontiguousarray(np.asarray(Wo, np.float32))
    bo_f = np.ascontiguousarray(np.asarray(bo, np.float32))

    nc = _get_nc()
    in_maps = []
    for c in range(N_CORES):
        b = c // (N_CORES // B)
        tq = c % (N_CORES // B)
        in_maps.append({
            "x_sl": np.ascontiguousarray(
                x[b, tq * T_CORE:(tq + 1) * T_CORE, :]),
            "adj_b": np.ascontiguousarray(adj[b]),
            "Wq": Wq_f, "bq": bq_f, "Wk": Wk_f, "bk": bk_f,
            "Wv": Wv_f, "bv": bv_f, "Wo": Wo_f, "bo": bo_f,
        })

    from concourse.bass_utils import run_bass_kernel_spmd
    res = run_bass_kernel_spmd(nc, in_maps, list(range(N_CORES)))

    out = np.empty((B, T, HID), np.float32)
    for c in range(N_CORES):
        b = c // (N_CORES // B)
        tq = c % (N_CORES // B)
        out[b, tq * T_CORE:(tq + 1) * T_CORE, :] = res.results[c]["out_sl"]
    return out
